# revision 13
# baseline (speedup 1.0000x reference)
"""Bezier surface fitter kernel for Trainium2 (8 NeuronCores, SPMD).

Math: out[b,c,i,j] = sum_{p,q} K[b,c,p,q] * basis[i,j,p,q]
The basis is separable: basis[i,j,p,q] = bu[i,p] * bv[j,q], so
    out_bc = bu @ K_bc @ bv^T
computed as two small GEMMs per (b,c):
    V_bc  = K_bc @ bv^T          (32x32 @ 32x256 -> 32x256)
    out_bc = bu @ V_bc           (256x32 @ 32x256 -> 256x256)
This is ~3.6 GFLOP total instead of the naive 103 GFLOP dense GEMM.

Precision: all matmuls run at bf16 PE speed but with fp32-equivalent
accuracy via hi/lo splitting (x = hi + lo, both bf16; products accumulate
in fp32 PSUM).  Stage 1 stacks all four hi/lo cross terms along the
contraction dim (k=4*32=128, exact).  Stage 2 accumulates the three
significant cross terms as 3 chained matmuls into one PSUM tile.
Expected rel err vs fp64 ~1e-5.

Sharding: data-parallel over batch b across 8 cores (32 batches each).
Stage-1 weights (K, host-pre-split/stacked/transposed) are the stationary
operand; stage-2 output tiles land in natural (i-partition, j-free) layout
so output DMA is fully contiguous 4 MB transfers.
"""

import numpy as np
from math import comb

NCORES = 8
B_, C_, H_, W_, M_, N_ = 256, 3, 256, 256, 31, 31
BSH = B_ // NCORES          # 32 batches per core
BC = BSH * C_               # 96 (b,c) pairs per core
NW = BC // 8                # 12 groups of 8 bc (one PSUM bank each)
WPG = 2                     # w-groups per DMA stage (16 bc = 4 MB)
NG = NW // WPG              # 6 stage groups

_CACHE = {}


def _bernstein(num_samples: int, deg: int) -> np.ndarray:
    t = (np.arange(num_samples, dtype=np.float64) + 0.5) / num_samples
    k = np.arange(deg + 1)
    coeff = np.array([comb(deg, i) for i in range(deg + 1)], dtype=np.float64)
    basis = coeff * (t[:, None] ** k) * ((1.0 - t[:, None]) ** (deg - k))
    return basis.astype(np.float32)


def _split_bf16(a: np.ndarray):
    import ml_dtypes
    hi = a.astype(ml_dtypes.bfloat16)
    lo = (a.astype(np.float32) - hi.astype(np.float32)).astype(ml_dtypes.bfloat16)
    return hi, lo


def _build_bass(reps: int = 1, mode: str = "full"):
    import concourse.bacc as bacc
    import concourse.mybir as mybir
    from concourse.tile import TileContext

    f32 = mybir.dt.float32
    bf16 = mybir.dt.bfloat16

    nc = bacc.Bacc("TRN2", target_bir_lowering=False)

    kt_d = nc.dram_tensor("kt", [128, BC * 32], bf16, kind="ExternalInput")
    bvt_d = nc.dram_tensor("bvt", [128, 256], bf16, kind="ExternalInput")
    buhi_d = nc.dram_tensor("buhi", [128, 256], bf16, kind="ExternalInput")
    bulo_d = nc.dram_tensor("bulo", [128, 256], bf16, kind="ExternalInput")
    out_d = nc.dram_tensor("out", [BC, 2, 128, 256], f32, kind="ExternalOutput")

    with TileContext(nc) as tc:
        with (
            tc.tile_pool(name="consts", bufs=1) as consts,
            tc.tile_pool(name="vsb", bufs=NW) as vsb,
            tc.tile_pool(name="vps", bufs=3, space="PSUM") as vps,
            tc.tile_pool(name="ops", bufs=4, space="PSUM") as ops,
            tc.tile_pool(name="stagep", bufs=2) as stagep,
        ):
            kt_all = consts.tile([128, BC * 32], bf16, name="kt_all")
            nc.sync.dma_start(out=kt_all, in_=kt_d[:, :])
            bvt_t = consts.tile([128, 256], bf16, name="bvt_t")
            nc.sync.dma_start(out=bvt_t, in_=bvt_d[:, :])
            buhi_t = consts.tile([128, 256], bf16, name="buhi_t")
            nc.sync.dma_start(out=buhi_t, in_=buhi_d[:, :])
            bulo_t = consts.tile([128, 256], bf16, name="bulo_t")
            nc.sync.dma_start(out=bulo_t, in_=bulo_d[:, :])

            def dma_only_pass(stage):
                for g in range(NG):
                    dma_dst = out_d[16 * g:16 * g + 16, :, :, :].rearrange(
                        "b h p j -> p b h j")
                    dma_src = stage.rearrange("p (b h j) -> p b h j",
                                              b=16, h=2)
                    nc.sync.dma_start(out=dma_dst, in_=dma_src)

            def one_pass():
                # ---- Phase A: V_bc = K_bc @ bvT (k=128 hi/lo stack) ----
                v_hi, v_lo = [], []
                for w in range(NW):
                    vp = vps.tile([128, 512], f32, name="vp", tag="vp")
                    for u in range(2):
                        for s in range(4):
                            bc = 8 * w + 4 * u + s
                            nc.tensor.matmul(
                                vp[32 * s:32 * s + 32, 256 * u:256 * u + 256],
                                lhsT=kt_all[:, 32 * bc:32 * bc + 32],
                                rhs=bvt_t[:, :],
                                start=True, stop=True,
                                tile_position=(0, 32 * s),
                            )
                    vhi = vsb.tile([128, 512], bf16, name="vhi", tag="vhi")
                    nc.vector.tensor_copy(vhi, vp)
                    vlo = vsb.tile([128, 512], bf16, name="vlo", tag="vlo")
                    nc.vector.tensor_sub(vlo, vp, vhi)
                    v_hi.append(vhi)
                    v_lo.append(vlo)

                # ---- Phase B: out_bc = bu @ V_bc (3-term hi/lo accum) ----
                ncopy = 0
                for g in range(NG):
                    stage = stagep.tile([128, WPG * 8 * 512], f32,
                                        name="stage", tag="stage")
                    for w in range(WPG * g, WPG * (g + 1)):
                        for h in range(2):
                            for s in range(4):
                                op = ops.tile([128, 512], f32, name="op",
                                              tag="op")
                                lh = (32 * s, 32 * s + 32)
                                hh = (128 * h, 128 * h + 128)
                                nc.tensor.matmul(
                                    op, lhsT=buhi_t[lh[0]:lh[1], hh[0]:hh[1]],
                                    rhs=v_hi[w][lh[0]:lh[1], :],
                                    start=True, stop=False,
                                    tile_position=(32 * s, 0))
                                nc.tensor.matmul(
                                    op, lhsT=bulo_t[lh[0]:lh[1], hh[0]:hh[1]],
                                    rhs=v_hi[w][lh[0]:lh[1], :],
                                    start=False, stop=False,
                                    tile_position=(32 * s, 0))
                                nc.tensor.matmul(
                                    op, lhsT=buhi_t[lh[0]:lh[1], hh[0]:hh[1]],
                                    rhs=v_lo[w][lh[0]:lh[1], :],
                                    start=False, stop=True,
                                    tile_position=(32 * s, 0))
                                # two bc outputs side by side in op
                                for u in range(2):
                                    bc = 8 * w + 4 * u + s
                                    b = bc - 16 * g
                                    dst = stage[:, b * 512 + h * 256:
                                                b * 512 + h * 256 + 256]
                                    src = op[:, 256 * u:256 * u + 256]
                                    if ncopy % 2 == 1:
                                        nc.scalar.copy(dst, src)
                                    else:
                                        nc.vector.tensor_copy(dst, src)
                                    ncopy += 1
                    if mode != "nodma":
                        dma_dst = out_d[16 * g:16 * g + 16, :, :, :].rearrange(
                            "b h p j -> p b h j")
                        dma_src = stage.rearrange("p (b h j) -> p b h j",
                                                  b=16, h=2)
                        nc.sync.dma_start(out=dma_dst, in_=dma_src)

            if mode == "dmaonly":
                stage0 = stagep.tile([128, WPG * 8 * 512], f32,
                                     name="stage0", tag="stage")
                nc.gpsimd.memset(stage0, 0.25)
                for _rep in range(reps):
                    dma_only_pass(stage0)
            else:
                for _rep in range(reps):
                    one_pass()

    nc.compile()
    return nc


def _get_bass(reps: int = 1, mode: str = "full"):
    key = f"nc{reps}:{mode}"
    if key not in _CACHE:
        _CACHE[key] = _build_bass(reps, mode)
    return _CACHE[key]


def _host_consts():
    if "consts" not in _CACHE:
        bu = _bernstein(H_, M_)            # (256, 32) [i, p] f32
        bv = _bernstein(W_, N_)            # (256, 32) [j, q] f32
        bvhi, bvlo = _split_bf16(np.ascontiguousarray(bv.T))   # (32, 256)
        bvt = np.ascontiguousarray(
            np.concatenate([bvhi, bvlo, bvhi, bvlo], axis=0))  # (128, 256)
        buhi, bulo = _split_bf16(np.ascontiguousarray(bu.T))   # (32, 256)
        buhi_r = np.ascontiguousarray(np.tile(buhi, (4, 1)))   # (128, 256)
        bulo_r = np.ascontiguousarray(np.tile(bulo, (4, 1)))   # (128, 256)
        _CACHE["consts"] = (bvt, buhi_r, bulo_r)
    return _CACHE["consts"]


def _host_prep_k(K_mat: np.ndarray):
    K = np.asarray(K_mat, dtype=np.float32)
    KT = K.transpose(0, 1, 3, 2)               # [b, c, q, p]
    KThi, KTlo = _split_bf16(KT)               # bf16 (256, 3, 32, 32)
    stack = np.concatenate([KThi, KThi, KTlo, KTlo], axis=2)  # (256,3,128,32)
    stack = stack.reshape(NCORES, BC, 128, 32)
    kt_host = np.ascontiguousarray(
        stack.transpose(0, 2, 1, 3)).reshape(NCORES, 128, BC * 32)
    return kt_host


def kernel(x: np.ndarray, K_mat: np.ndarray) -> np.ndarray:
    from concourse.bass_utils import run_bass_kernel_spmd

    nc = _get_bass()
    bvt, buhi_r, bulo_r = _host_consts()
    kt_host = _host_prep_k(K_mat)

    in_maps = [
        {"kt": np.ascontiguousarray(kt_host[d]), "bvt": bvt,
         "buhi": buhi_r, "bulo": bulo_r}
        for d in range(NCORES)
    ]
    res = run_bass_kernel_spmd(nc, in_maps, core_ids=list(range(NCORES)))
    _CACHE["last_results"] = res

    parts = [r["out"].reshape(BSH, C_, H_, W_) for r in res.results]
    full = np.concatenate(parts, axis=0)  # (256, 3, 256, 256)
    return full[None]


# revision 37
# speedup vs baseline: 1.5356x; 1.5356x over previous
"""Bezier surface fitter kernel for Trainium2 (8 NeuronCores, SPMD).

Math: out[b,c,i,j] = sum_{p,q} K[b,c,p,q] * basis[i,j,p,q]
The basis is separable: basis[i,j,p,q] = bu[i,p] * bv[j,q], so
    out_bc = bu @ K_bc @ bv^T
computed as two small GEMMs per (b,c):
    V_bc  = K_bc @ bv^T          (32x32 @ 32x256 -> 32x256)
    out_bc = bu @ V_bc           (256x32 @ 32x256 -> 256x256)
This is ~3.6 GFLOP total instead of the naive 103 GFLOP dense GEMM.

Precision: matmuls run at fp16 PE speed (1 cycle/row) with fp32-level
accuracy via hi/lo splitting (x = hi + lo, both fp16; products are exact
in fp32 and accumulate in fp32 PSUM):
  - Stage 1 stacks all four K/bv hi/lo cross terms along the contraction
    dim (k = 4*32 = 128) -> V is exact, and the stationary K operand has
    its 32 columns duplicated so V lands DUPLICATED on two 32-partition
    slots (m=64).
  - Stage 2 contracts k=64 against stacked [bu_hi; bu_lo] weights: one
    matmul computes bu_hi@Vx + bu_lo@Vx; two accumulating matmuls (Vhi,
    Vlo) give the full 4-term product. Expected rel err ~1e-6.

Sharding: data-parallel over batch b across 8 cores (32 batches each).
Stage-2 output tiles land in natural (i-partition, j-free) layout so the
output DMA is 12 fully-contiguous 2 MB transfers that stream gapless.
"""

import numpy as np
from math import comb

NCORES = 8
B_, C_, H_, W_, M_, N_ = 256, 3, 256, 256, 31, 31
BSH = B_ // NCORES          # 32 batches per core
BC = BSH * C_               # 96 (b,c) pairs per core
NV = BC // 4                # 24 v-tiles of 4 bc each
VPG = 2                     # v-tiles per DMA stage (8 bc = 2 MB)
NG = NV // VPG              # 12 stage groups
NB = 4 * VPG                # bc per stage
DVE_MOD = 2                 # 1 of every DVE_MOD copies goes to DVE (rest ACT)

_CACHE = {}


def _bernstein(num_samples: int, deg: int) -> np.ndarray:
    t = (np.arange(num_samples, dtype=np.float64) + 0.5) / num_samples
    k = np.arange(deg + 1)
    coeff = np.array([comb(deg, i) for i in range(deg + 1)], dtype=np.float64)
    basis = coeff * (t[:, None] ** k) * ((1.0 - t[:, None]) ** (deg - k))
    return basis.astype(np.float32)


def _split_fp16(a: np.ndarray):
    hi = a.astype(np.float16)
    lo = (a.astype(np.float32) - hi.astype(np.float32)).astype(np.float16)
    return hi, lo


AIN = 512 + BC * 64  # [bvt(256) | bustack(256) | kt(BC*64)]


def _build_bass(reps: int = 1, mode: str = "full"):
    import concourse.bacc as bacc
    import concourse.mybir as mybir
    from concourse.tile import TileContext

    f32 = mybir.dt.float32
    f16 = mybir.dt.float16

    nc = bacc.Bacc("TRN2", target_bir_lowering=False)

    allin_d = nc.dram_tensor("allin", [128, AIN], f16, kind="ExternalInput")
    out_d = nc.dram_tensor("out", [BC, 2, 128, 256], f32, kind="ExternalOutput")

    with TileContext(nc) as tc:
        with (
            tc.tile_pool(name="consts", bufs=1) as consts,
            tc.tile_pool(name="vsb", bufs=NV) as vsb,
            tc.tile_pool(name="vps", bufs=3, space="PSUM") as vps,
            tc.tile_pool(name="ops", bufs=4, space="PSUM") as ops,
            tc.tile_pool(name="stagep", bufs=4) as stagep,
        ):
            allin = consts.tile([128, AIN], f16, name="allin_t")
            # chunked input load: consts + first v-tiles first
            bounds = [0, 1024, 3840, AIN]
            for ch in range(len(bounds) - 1):
                nc.sync.dma_start(
                    out=allin[:, bounds[ch]:bounds[ch + 1]],
                    in_=allin_d[:, bounds[ch]:bounds[ch + 1]])
            bvt_t = allin[:, 0:256]
            bust_t = allin[:, 256:512]   # [buhiT; buloT; buhiT; buloT]
            kt_all = allin[:, 512:AIN]   # per bc: (128, 64), cols duplicated

            # PE warm-up: dummy matmuls during the input-load window keep
            # the HAM activity monitor busy so real matmuls ramp up sooner.
            warm = consts.tile([128, 128], f16, name="warm")
            nc.gpsimd.memset(warm, 0.0)
            wps = vps.tile([128, 512], f32, name="wps", tag="vp")
            for _ in range(10):
                nc.tensor.matmul(wps[0:128, 0:128], lhsT=warm, rhs=warm,
                                 start=True, stop=True)

            def stage_dma(stage, g, split=False):
                # stage free layout: [bc_local(NB), h(2), j(256)]
                if split:
                    for h in range(2):
                        dma_dst = out_d[NB * g:NB * (g + 1), h, :, :].rearrange(
                            "b p j -> p b j")
                        dma_src = stage.rearrange(
                            "p (b h j) -> p b h j", b=NB, h=2)[:, :, h, :]
                        nc.sync.dma_start(out=dma_dst, in_=dma_src)
                else:
                    dma_dst = out_d[NB * g:NB * (g + 1), :, :, :].rearrange(
                        "b h p j -> p b h j")
                    dma_src = stage.rearrange("p (b h j) -> p b h j", b=NB, h=2)
                    nc.sync.dma_start(out=dma_dst, in_=dma_src)

            def phase_a(v):
                # V of bc = 4v + 2*s2 + u lands at vp[64*s2:, 256*u:],
                # duplicated across the two 32-partition halves of its slot.
                vp = vps.tile([128, 512], f32, name="vp", tag="vp")
                for s2 in range(2):
                    for u in range(2):
                        bc = 4 * v + 2 * s2 + u
                        nc.tensor.matmul(
                            vp[64 * s2:64 * s2 + 64, 256 * u:256 * u + 256],
                            lhsT=kt_all[:, 64 * bc:64 * bc + 64],
                            rhs=bvt_t[:, :],
                            start=True, stop=True,
                            tile_position=(0, 64 * s2),
                        )
                vhi = vsb.tile([128, 512], f16, name="vhi", tag="vhi")
                nc.scalar.copy(vhi, vp)
                vlo = vsb.tile([128, 512], f16, name="vlo", tag="vlo")
                nc.vector.tensor_sub(vlo, vp, vhi)
                return vhi, vlo

            state = {"ncopy": 0}

            def phase_b(v, g, stage, vhi, vlo):
                vb = v - VPG * g
                for h in range(2):
                    for s2 in range(2):
                        op = ops.tile([128, 512], f32, name="op", tag="op")
                        lh = (64 * s2, 64 * s2 + 64)
                        hh = (128 * h, 128 * h + 128)
                        nc.tensor.matmul(
                            op, lhsT=bust_t[lh[0]:lh[1], hh[0]:hh[1]],
                            rhs=vhi[lh[0]:lh[1], :],
                            start=True, stop=False,
                            tile_position=(64 * s2, 0))
                        nc.tensor.matmul(
                            op, lhsT=bust_t[lh[0]:lh[1], hh[0]:hh[1]],
                            rhs=vlo[lh[0]:lh[1], :],
                            start=False, stop=True,
                            tile_position=(64 * s2, 0))
                        # op free halves are bc = 4v+2*s2 (u=0), +1 (u=1);
                        # stage free = bc*512 + h*256 + j -> u-stride 512.
                        sv = stage.rearrange(
                            "p (q u h j) -> p q u h j", q=2 * VPG, u=2, h=2)
                        dst = sv[:, 2 * vb + s2, :, h, :]
                        src = op.rearrange("p (u j) -> p u j", u=2)
                        if state["ncopy"] % DVE_MOD == 0:
                            nc.vector.tensor_copy(dst, src)
                        else:
                            nc.scalar.copy(dst, src)
                        state["ncopy"] += 1

            def one_pass():
                for g in range(NG):
                    stage = stagep.tile([128, NB * 512], f32,
                                        name="stage", tag="stage")
                    vs = [phase_a(v) for v in range(VPG * g, VPG * (g + 1))]
                    for i, v in enumerate(range(VPG * g, VPG * (g + 1))):
                        phase_b(v, g, stage, *vs[i])
                    if mode != "nodma":
                        stage_dma(stage, g, split=(g == 0 or g == NG - 1))

            if mode == "dmaonly":
                stage0 = stagep.tile([128, NB * 512], f32,
                                     name="stage0", tag="stage")
                nc.gpsimd.memset(stage0, 0.25)
                for _rep in range(reps):
                    for g in range(NG):
                        stage_dma(stage0, g)
            else:
                for _rep in range(reps):
                    one_pass()

    nc.compile()
    return nc


def _get_bass(reps: int = 1, mode: str = "full"):
    key = f"nc{reps}:{mode}"
    if key not in _CACHE:
        _CACHE[key] = _build_bass(reps, mode)
    return _CACHE[key]


def _host_consts():
    if "consts" not in _CACHE:
        bu = _bernstein(H_, M_)            # (256, 32) [i, p] f32
        bv = _bernstein(W_, N_)            # (256, 32) [j, q] f32
        bvhi, bvlo = _split_fp16(np.ascontiguousarray(bv.T))   # (32, 256)
        bvt = np.ascontiguousarray(
            np.concatenate([bvhi, bvlo, bvhi, bvlo], axis=0))  # (128, 256)
        buhi, bulo = _split_fp16(np.ascontiguousarray(bu.T))   # (32, 256)
        bust = np.ascontiguousarray(
            np.concatenate([buhi, bulo, buhi, bulo], axis=0))  # (128, 256)
        _CACHE["consts"] = (bvt, bust)
    return _CACHE["consts"]


def _host_prep_k(K_mat: np.ndarray):
    K = np.asarray(K_mat, dtype=np.float32)
    KT = K.transpose(0, 1, 3, 2)               # [b, c, q, p]
    KThi, KTlo = _split_fp16(KT)               # fp16 (256, 3, 32, 32)
    stack = np.concatenate([KThi, KThi, KTlo, KTlo], axis=2)  # (256,3,128,32)
    stack = np.concatenate([stack, stack], axis=3)            # dup cols -> 64
    stack = stack.reshape(NCORES, BC, 128, 64)
    kt_host = np.ascontiguousarray(
        stack.transpose(0, 2, 1, 3)).reshape(NCORES, 128, BC * 64)
    return kt_host


def _make_in_maps(K_mat: np.ndarray):
    bvt, bust = _host_consts()
    kt_host = _host_prep_k(K_mat)
    consts = np.concatenate([bvt, bust], axis=1).astype(np.float16)
    return [
        {"allin": np.ascontiguousarray(
            np.concatenate([consts, kt_host[d]], axis=1))}
        for d in range(NCORES)
    ]


def kernel(x: np.ndarray, K_mat: np.ndarray) -> np.ndarray:
    from concourse.bass_utils import run_bass_kernel_spmd

    nc = _get_bass()
    in_maps = _make_in_maps(K_mat)
    res = run_bass_kernel_spmd(nc, in_maps, core_ids=list(range(NCORES)))
    _CACHE["last_results"] = res

    parts = [r["out"].reshape(BSH, C_, H_, W_) for r in res.results]
    full = np.concatenate(parts, axis=0)  # (256, 3, 256, 256)
    return full[None]
